# revision 44
# baseline (speedup 1.0000x reference)
"""MoE (top-2 of 8 experts) Trainium2 kernel — fp8 DoubleRow version.

Strategy: expert-parallel across the 8 NeuronCores (host routes tokens,
core e computes expert e's MLP over its gathered tokens). The matmuls run
in fp8(e4m3) DoubleRow mode — one DR instruction contracts TWO 128-row
k-tiles in 0.5 cycles per output column (4x the fp32r rate) — with a
hi/lo split-correction that keeps the end-to-end relative error ~2e-3:

  operand a is stored as a_hi = e4m3(a) and a_lo = e4m3(a - a_hi); the
  product a·w is assembled from three rank-K products
      a_hi·w_hi + a_hi·w_lo + a_lo·w_hi       (a_lo·w_lo ~ 2^-8, dropped)
  The DR pair slots compute two rank-128 products per instruction:
    - "plain"  pairs two k-tiles of (a_hi, w_hi): the main term,
    - "paired" puts (w_hi, w_lo) against (a_lo, a_hi) of ONE k-tile: both
      correction terms in one instruction.
  Stage 1 (contraction H=1024, 8 k-tiles): 4 plain + 8 paired = 6 cyc/col
  Stage 2 (contraction I=1408, 11 k-tiles): 6 plain (one zero-padded) +
      9 paired = 7.5 cyc/col
  vs fp32r's 8 and 11 cyc/col, and the fp8 operands halve the DMA bytes.
  Stage 2 drops the correction products for k-tiles it9/it10 (NCORR2=9):
  leaving 2 of 11 stage-2 k-tiles uncorrected raises the end-to-end error
  from 2.5e-3 to ~1.6e-2 (still under the 2e-2 gate) and saves 8 cyc/col
  (~3.5us of PE time at count~1058).

Scaling: w1 is host-scaled by SW1=32 (so its lo-part stays in e4m3 normal
range), making psum1 = 32·z. Sigmoid reads psum with scale 1/32; the DVE
multiply gives hv = 32·silu(z) (absmax ~212 < e4m3 max 240), which is
split hi/lo for stage 2. w2 is scaled by SW2=32 and the host pre-divides
the gates by SW1·SW2 so the stage-2 gate-multiply absorbs all scales.

DMA orchestration (the cost model serializes descriptor generation at
~625ns per dma_start and all copies on one engine at 0.3855 ns per
byte-per-partition, with a 2x penalty when contiguous runs are < 512B):
  - column chunks are [512, 512, tail] so every x / y slice moves in
    >=512B contiguous runs and each psum group fills a whole 2KB bank;
  - ~20 input DMAs, each ~>=625ns of copy, ordered to track the PE's
    consumption: w1[it0], x-c0 hi, x-c0 lo interleaved with w1[it1..],
    then x-c1 mid-stream, then gates/w2, then the x tail;
  - y is accumulated per chunk in SBUF ([P, HK, cols] bf16) and written
    with ONE descriptor per non-final chunk; the final chunk writes
    per-ht slivers so the post-matmul drain is minimal.

Per-core device pipeline (count = max tokens routed to one expert):
  stage 1, chunk-outer: psum[it] group (full 2KB bank, two 256-col DR
    half-sweeps) -> ACT sigmoid -> DVE mul (hv) -> ACT copy-cast (h_hi)
    -> GpSimd sub (h_lo), writing h into hlh [p, slot(lo,hi,zero), it, C]
  stage 2: psum[ht] group -> DVE gate-mul -> ys chunk buffer -> DMA out
    yT [H, C] bf16.  Software pipeline: s2 of chunk i runs after s1 of
    chunk i+1 so the PE stays fed while evac chains drain.
The host transposes and scatter-adds the two expert contributions.
"""

import numpy as np
import ml_dtypes

import concourse.mybir as mybir
from concourse import bacc
from concourse.tile import TileContext
from concourse.bass_utils import run_bass_kernel_spmd

T, H, I, E = 4096, 1024, 1408, 8
TOPK = 2
P = 128
HK = H // P  # 8
IT = I // P  # 11
N_CORES = 8
F32 = mybir.dt.float32
F8 = mybir.dt.float8e4
E4 = ml_dtypes.float8_e4m3
AF = mybir.ActivationFunctionType
DR = mybir.MatmulPerfMode.DoubleRow
SW1 = 32.0
SW2 = 32.0
NCORR2 = IT - 2  # stage-2 k-tiles that get the hi/lo correction products

# most recently built device program (for test harnesses / cost-model timing)
LAST_NC = None


def _chunks(count):
    """512-wide column chunks (one full PSUM bank each) plus a tail.
    512-wide chunks keep x/y DMA contiguous runs >= 512B (full-rate DMA)."""
    out = []
    rem = count
    while rem > 0:
        c = min(512, rem)
        out.append(c)
        rem -= c
    return out


def _halves(cs):
    """Split a chunk into DR-sized half-sweeps (moving free dim 2*cols must
    stay <= 512, so <= 256 output columns per DR matmul); halves stay even."""
    if cs <= 256:
        return [(0, cs)]
    h0 = (cs // 2 + 1) // 2 * 2
    return [(0, h0), (h0, cs - h0)]


def build_moe_expert_kernel(count):
    """One-expert MLP over `count` gathered tokens (even)."""
    C = count
    assert count % 2 == 0
    nc = bacc.Bacc("TRN2", target_bir_lowering=False, debug=False, num_devices=N_CORES)

    xlh_d = nc.dram_tensor("xlh", [P, 2 * HK * C], F8, kind="ExternalInput").ap()
    w1_d = nc.dram_tensor("whl1", [P, IT * 2 * HK * P], F8, kind="ExternalInput").ap()
    w2_d = nc.dram_tensor("whl2", [P, HK * IT * 2 * P], F8, kind="ExternalInput").ap()
    g_d = nc.dram_tensor("gates", [1, C], F32, kind="ExternalInput").ap()
    y_d = nc.dram_tensor("yT", [H, C], mybir.dt.bfloat16, kind="ExternalOutput").ap()
    # The final chunk (<= 64 cols) runs stage 2 fused: all 8 ht groups
    # accumulate into ONE psum bank (ht-blocked columns), one gate-mul over
    # host-replicated gates, one contiguous flush into its own tensor
    # [p, (ht, col)] that the host stitches back into column position.
    ctail = _chunks(count)[-1]
    fused_tail = ctail <= 64
    yt_d = nc.dram_tensor(
        "yTail", [P, HK * ctail], mybir.dt.bfloat16, kind="ExternalOutput"
    ).ap()
    gt_d = nc.dram_tensor("gates_tail", [1, HK * ctail], F32, kind="ExternalInput").ap()

    # logical views (slot order: w (hi, lo); x and h (lo, hi[, zero]))
    xlh_v = xlh_d.rearrange("p (s k c) -> p s k c", s=2, k=HK)
    w1_v = w1_d.rearrange("p (i s k j) -> p i s k j", i=IT, s=2, k=HK)
    w2_v = w2_d.rearrange("p (h i s j) -> p h i s j", h=HK, i=IT, s=2)
    y_v = y_d.rearrange("(h p) c -> p h c", p=P)  # [128, HK, C]

    cks = _chunks(count)
    c_starts = [sum(cks[:j]) for j in range(len(cks))]
    cl = list(zip(c_starts, cks))

    with TileContext(nc) as tc:
        with (
            tc.tile_pool(name="w", bufs=1) as wpool,
            tc.tile_pool(name="hv", bufs=3) as hvpool,
            tc.tile_pool(name="y", bufs=2) as ypool,
            tc.tile_pool(name="ps1", bufs=5, space="PSUM") as ps1p,
            tc.tile_pool(name="ps2", bufs=3, space="PSUM") as ps2p,
        ):
            wt1 = wpool.tile([P, IT, 2, HK, P], F8)
            wt2 = wpool.tile([P, HK, IT, 2, P], F8)
            xt = wpool.tile([P, 2, HK, C], F8)
            hlh = wpool.tile([P, 3, IT, C], F8)
            gb = wpool.tile([P, C], F32)

            # ---- input DMA stream (single SP HWDGE queue; issue order =
            # serial copy order = PE consumption order) ----
            def ldx(s, k0, k1, a, b):
                nc.sync.dma_start(xt[:, s, k0:k1, a:b], xlh_v[:, s, k0:k1, a:b])

            a0, b0 = cl[0][0], cl[0][0] + cl[0][1]
            nc.sync.dma_start(wt1[:, 0], w1_v[:, 0])  # w1 it0 (hi+lo)
            ldx(1, 0, 4, a0, b0)                      # x chunk0 hi hk0:4
            ldx(1, 4, 8, a0, b0)                      # x chunk0 hi hk4:8
            ldx(0, 0, 4, a0, b0)                      # x chunk0 lo hk0:4
            ldx(0, 4, 8, a0, b0)                      # x chunk0 lo hk4:8
            for it in range(1, 7):
                nc.sync.dma_start(wt1[:, it], w1_v[:, it])
            if len(cl) > 1:                           # x chunk1 mid-stream
                a1, b1 = cl[1][0], cl[1][0] + cl[1][1]
                ldx(1, 0, 4, a1, b1)
                ldx(1, 4, 8, a1, b1)
                ldx(0, 0, 4, a1, b1)
                ldx(0, 4, 8, a1, b1)
            for it in range(7, IT):
                nc.sync.dma_start(wt1[:, it], w1_v[:, it])
            nc.sync.dma_start(gb[:], g_d[0].partition_broadcast(P))
            gbt = wpool.tile([P, HK * ctail], F32)
            if fused_tail:
                nc.sync.dma_start(gbt[:], gt_d[0].partition_broadcast(P))
            for ht in range(HK):
                nc.sync.dma_start(wt2[:, ht], w2_v[:, ht])
            for c0, cs in cl[2:]:                     # x tail chunk(s)
                nc.sync.dma_start(
                    xt[:, :, :, c0 : c0 + cs], xlh_v[:, :, :, c0 : c0 + cs]
                )

            # PE warm-up: dummy DR matmuls on memset scratch keep the tensor
            # engine busy from ~0.9us while the first DMAs land, so the
            # p-state ramp (full clock only after ~3us of PE activity)
            # completes before the real matmul stream begins. The scratch
            # memsets go on Pool so they don't queue behind DVE work.
            wmw = wpool.tile([P, 2, P], F8)
            wmx = wpool.tile([P, 2, 256], F8)
            nc.gpsimd.memset(wmw[:], 0.0)
            nc.gpsimd.memset(wmx[:], 0.0)
            wps = ps2p.tile([P, 512], F32, tag="ps2")
            NWARM = 38
            for r in range(NWARM):
                nc.tensor.matmul(
                    wps[:, 0:256],
                    wmw[:],
                    wmx[:],
                    start=(r == 0),
                    stop=(r == NWARM - 1),
                    perf_mode=DR,
                )

            # the only zero-slot region stage 2 ever reads (it10 plain term)
            nc.vector.memset(hlh[:, 2, IT - 1, :], 0.0)

            def s1_plains(it, c0, cs):
                # plains (both halves) first: they only need the hi slots,
                # which the DMA stream delivers before the lo slots
                ps = ps1p.tile([P, 512], F32, tag="ps1")
                for h0, hcs in _halves(cs):
                    a, b = c0 + h0, c0 + h0 + hcs
                    for hkp in range(0, HK, 2):  # plain: x_hi @ w1_hi
                        nc.tensor.matmul(
                            ps[:, h0 : h0 + hcs],
                            wt1[:, it, 0, hkp : hkp + 2, :],
                            xt[:, 1, hkp : hkp + 2, a:b],
                            start=(h0 == 0 and hkp == 0),
                            stop=False,
                            perf_mode=DR,
                        )
                return ps

            def s1_rest(it, c0, cs, ps):
                for h0, hcs in _halves(cs):
                    a, b = c0 + h0, c0 + h0 + hcs
                    for hk in range(HK):  # paired: w_hi*x_lo + w_lo*x_hi
                        nc.tensor.matmul(
                            ps[:, h0 : h0 + hcs],
                            wt1[:, it, :, hk, :],
                            xt[:, :, hk, a:b],
                            start=False,
                            stop=(h0 + hcs == cs and hk == HK - 1),
                            perf_mode=DR,
                        )
                # evacuate: hv = psum * sigmoid(psum/SW1) = SW1*silu(z),
                # then split h into e4m3 hi/lo for stage 2
                sg = hvpool.tile([P, 512], F32, tag="sg")
                nc.scalar.activation(
                    sg[:, :cs], ps[:, :cs], AF.Sigmoid, scale=1.0 / SW1
                )
                hv = hvpool.tile([P, 512], F32, tag="hv")
                nc.vector.tensor_mul(out=hv[:, :cs], in0=ps[:, :cs], in1=sg[:, :cs])
                nc.scalar.activation(hlh[:, 1, it, c0 : c0 + cs], hv[:, :cs], AF.Copy)
                if it < NCORR2:  # h_lo is only read by corrected stage-2 tiles
                    nc.gpsimd.tensor_sub(
                        hlh[:, 0, it, c0 : c0 + cs],
                        hv[:, :cs],
                        hlh[:, 1, it, c0 : c0 + cs],
                    )

            def s2_chunk(ci, extras=()):
                """Stage 2 for chunk ci: 8 ht psum groups -> gate-mul into a
                per-chunk ys buffer. `extras[ht]` (thunks) are interleaved
                after each ht group — used to hide the tail chunk's tiny
                stage-1 groups (and their evac-latency psum stalls) behind
                this chunk's large stage-2 groups."""
                c0, cs = cl[ci]
                last = ci == len(cl) - 1
                # exact-size buffer for the final chunk keeps its single
                # flush DMA contiguous (>=512B runs, no 2x DMA penalty)
                ys = ypool.tile(
                    [P, HK, cs if last else 512],
                    mybir.dt.bfloat16,
                    tag="ys_tail" if last else "ys",
                )
                for ht in range(HK):
                    # the final (small) chunk draws psum tiles from BOTH pools
                    # (stage 1 is done with ps1 by then) so its 8 back-to-back
                    # groups never wait on gate-mul drains to free a bank
                    pool = (ps1p if ht % 2 == 0 else ps2p) if last else ps2p
                    ps = pool.tile([P, 512], F32, tag="ps1" if pool is ps1p else "ps2")
                    for h0, hcs in _halves(cs):
                        a, b = c0 + h0, c0 + h0 + hcs
                        for itp in range(0, IT - 1, 2):  # plain: h_hi @ w2_hi
                            nc.tensor.matmul(
                                ps[:, h0 : h0 + hcs],
                                wt2[:, ht, itp : itp + 2, 0, :],
                                hlh[:, 1, itp : itp + 2, a:b],
                                start=(h0 == 0 and itp == 0),
                                stop=False,
                                perf_mode=DR,
                            )
                        # it10 plain, zero-padded second slot
                        nc.tensor.matmul(
                            ps[:, h0 : h0 + hcs],
                            wt2[:, ht, IT - 1, :, :],
                            hlh[:, 1:3, IT - 1, a:b],
                            start=False,
                            stop=False,
                            perf_mode=DR,
                        )
                        for it in range(NCORR2):  # paired: w2_hi*h_lo + w2_lo*h_hi
                            nc.tensor.matmul(
                                ps[:, h0 : h0 + hcs],
                                wt2[:, ht, it, :, :],
                                hlh[:, 0:2, it, a:b],
                                start=False,
                                stop=(h0 + hcs == cs and it == NCORR2 - 1),
                                perf_mode=DR,
                            )
                    nc.vector.tensor_mul(
                        out=ys[:, ht, :cs], in0=ps[:, :cs], in1=gb[:, c0 : c0 + cs]
                    )
                    # big chunks: flush y per 2 ht so copies stream out while
                    # later ht groups still compute (keeps the big copy off
                    # the kernel tail); the LAST flush is ht7 alone so the
                    # final copy in flight is small.
                    if not last and (ht % 2 == 1 or ht >= 6):
                        f0 = ht if ht >= 6 else ht - 1
                        nc.sync.dma_start(
                            y_v[:, f0 : ht + 1, c0 : c0 + cs],
                            ys[:, f0 : ht + 1, :cs],
                        )
                    if ht < len(extras):
                        extras[ht]()
                if last:
                    nc.sync.dma_start(y_v[:, :, c0 : c0 + cs], ys[:, :, :cs])

            def s2_tail_fused():
                """Stage 2 for the final small chunk, fused: all 8 ht groups
                accumulate into ONE psum bank at ht-blocked column offsets,
                then a single gate-mul (host-replicated gates) and a single
                fully-contiguous flush — minimal post-matmul drain."""
                c0, cs = cl[-1]
                ps = ps1p.tile([P, 512], F32, tag="ps1")
                ys = ypool.tile([P, HK * ctail], mybir.dt.bfloat16, tag="ys_f")
                for ht in range(HK):
                    if ht == 6:
                        # gate-mul ht0-5 while ht6/7 matmuls still run: only
                        # a 2-ht-wide multiply trails the last matmul
                        nc.vector.tensor_mul(
                            out=ys[:, : 6 * cs],
                            in0=ps[:, : 6 * cs],
                            in1=gbt[:, : 6 * cs],
                        )
                    o = ht * cs
                    for itp in range(0, IT - 1, 2):
                        nc.tensor.matmul(
                            ps[:, o : o + cs],
                            wt2[:, ht, itp : itp + 2, 0, :],
                            hlh[:, 1, itp : itp + 2, c0 : c0 + cs],
                            start=(itp == 0),
                            stop=False,
                            perf_mode=DR,
                        )
                    nc.tensor.matmul(
                        ps[:, o : o + cs],
                        wt2[:, ht, IT - 1, :, :],
                        hlh[:, 1:3, IT - 1, c0 : c0 + cs],
                        start=False,
                        stop=False,
                        perf_mode=DR,
                    )
                    for it in range(NCORR2):
                        nc.tensor.matmul(
                            ps[:, o : o + cs],
                            wt2[:, ht, it, :, :],
                            hlh[:, 0:2, it, c0 : c0 + cs],
                            start=False,
                            stop=(it == NCORR2 - 1),
                            perf_mode=DR,
                        )
                nc.vector.tensor_mul(
                    out=ys[:, 6 * cs :],
                    in0=ps[:, 6 * cs : HK * cs],
                    in1=gbt[:, 6 * cs :],
                )
                nc.sync.dma_start(yt_d, ys[:])

            # chunk 0: staggered — run DEPTH groups' plains ahead so the PE
            # has hi-slot work while the lo slots / later w1 slices stream in
            DEPTH = 4
            c0_, cs_ = cl[0]
            pss = {}
            for it in range(min(DEPTH, IT)):
                pss[it] = s1_plains(it, c0_, cs_)
            for it in range(IT):
                s1_rest(it, c0_, cs_, pss.pop(it))
                if it + DEPTH < IT:
                    pss[it + DEPTH] = s1_plains(it + DEPTH, c0_, cs_)
            # software pipeline: stage-2 of chunk i runs after stage-1 of
            # chunk i+1, so the PE stays fed while evac chains drain. The
            # final (small) chunk's stage-1 groups are interleaved INTO the
            # previous chunk's stage-2 sweep — their evac-latency psum waits
            # hide behind the big ht groups instead of stalling the PE.
            for ci in range(1, len(cl) - 1):
                c0_, cs_ = cl[ci]
                for it in range(IT):
                    s1_rest(it, c0_, cs_, s1_plains(it, c0_, cs_))
                s2_chunk(ci - 1)
            if len(cl) > 1:
                ct_, cst_ = cl[-1]

                def tail_s1(it):
                    return lambda: s1_rest(it, ct_, cst_, s1_plains(it, ct_, cst_))

                per_ht = [[] for _ in range(HK)]
                for it in range(IT):
                    per_ht[min(it // 2, HK - 1)].append(it)

                def slot(hts):
                    return lambda: [tail_s1(it)() for it in hts]

                s2_chunk(len(cl) - 2, extras=[slot(h) for h in per_ht])
            if fused_tail:
                s2_tail_fused()
            else:
                s2_chunk(len(cl) - 1)

    nc.compile()
    global LAST_NC
    LAST_NC = nc
    return nc


def route(router_logits):
    """Host-side router: softmax -> top-2 -> renormalize."""
    logits = np.asarray(router_logits, dtype=np.float32)
    m = logits.max(axis=-1, keepdims=True)
    ex = np.exp(logits - m)
    probs = ex / ex.sum(axis=-1, keepdims=True)
    order = np.argsort(-probs, axis=-1, kind="stable")[:, :TOPK]
    rows = np.arange(logits.shape[0])[:, None]
    topk_p = probs[rows, order]
    topk_p = topk_p / topk_p.sum(axis=-1, keepdims=True)
    return order, topk_p.astype(np.float32)


def _q8(a):
    return np.asarray(a, dtype=np.float32).astype(E4)


def kernel(x, router_logits, w1, w2):
    x = np.ascontiguousarray(np.asarray(x, dtype=np.float32))
    w1 = np.asarray(w1, dtype=np.float32)
    w2 = np.asarray(w2, dtype=np.float32)
    t = x.shape[0]

    top2_idx, top2_gate = route(router_logits)

    expert_tokens = []
    expert_gates = []
    for e in range(E):
        sel = np.nonzero(top2_idx == e)
        expert_tokens.append(sel[0])
        expert_gates.append(top2_gate[sel[0], sel[1]])
    counts = [len(ix) for ix in expert_tokens]
    count = max(2, max(counts) + max(counts) % 2)

    nc = build_moe_expert_kernel(count)

    in_maps = []
    for e in range(E):
        cnt = counts[e]
        xe = x[expert_tokens[e]]  # [cnt, H]
        x_hi = _q8(xe)
        x_lo = _q8(xe - x_hi.astype(np.float32))
        xlh = np.zeros((P, 2, HK, count), dtype=E4)
        xlh[:, 0, :, :cnt] = x_lo.reshape(cnt, HK, P).transpose(2, 1, 0)
        xlh[:, 1, :, :cnt] = x_hi.reshape(cnt, HK, P).transpose(2, 1, 0)

        W1 = SW1 * w1[e]  # [I, H]
        W1_hi = _q8(W1)
        W1_lo = _q8(W1 - W1_hi.astype(np.float32))
        # whl1[p, it, slot, hk, j] = W1_s[it*128+j, hk*128+p]
        w1hi_t = W1_hi.reshape(IT, P, HK, P).transpose(3, 0, 2, 1)
        w1lo_t = W1_lo.reshape(IT, P, HK, P).transpose(3, 0, 2, 1)
        whl1 = np.stack([w1hi_t, w1lo_t], axis=2)  # [p, it, 2, hk, j]

        W2 = SW2 * w2[e]  # [H, I]
        W2_hi = _q8(W2)
        W2_lo = _q8(W2 - W2_hi.astype(np.float32))
        # whl2[p, ht, it, slot, j] = W2_s[ht*128+j, it*128+p]
        w2hi_t = W2_hi.reshape(HK, P, IT, P).transpose(3, 0, 2, 1)
        w2lo_t = W2_lo.reshape(HK, P, IT, P).transpose(3, 0, 2, 1)
        whl2 = np.stack([w2hi_t, w2lo_t], axis=3)  # [p, ht, it, 2, j]

        g = np.zeros((1, count), dtype=np.float32)
        g[0, :cnt] = expert_gates[e] / (SW1 * SW2)
        ctail = _chunks(count)[-1]
        gt = np.tile(g[:, count - ctail :], (1, HK))  # [1, HK*ctail]

        in_maps.append(
            {
                "xlh": np.ascontiguousarray(xlh).reshape(P, -1),
                "whl1": np.ascontiguousarray(whl1).reshape(P, -1),
                "whl2": np.ascontiguousarray(whl2).reshape(P, -1),
                "gates": g,
                "gates_tail": np.ascontiguousarray(gt),
            }
        )

    res = run_bass_kernel_spmd(nc, in_maps, core_ids=list(range(N_CORES)))
    if not all(np.isfinite(r["yT"]).all() for r in res.results):
        # one retry in case of a transient device fault
        res = run_bass_kernel_spmd(nc, in_maps, core_ids=list(range(N_CORES)))

    ctail = _chunks(count)[-1]
    fused_tail = ctail <= 64
    out = np.zeros((t, H), dtype=np.float32)
    for e in range(E):
        cnt = counts[e]
        yT = res.results[e]["yT"]
        if fused_tail:
            # yTail[p, ht*ctail + c] holds row ht*128+p, col count-ctail+c
            ytl = (
                res.results[e]["yTail"]
                .reshape(P, HK, ctail)
                .transpose(1, 0, 2)
                .reshape(H, ctail)
            )
            yT = np.concatenate([yT[:, : count - ctail], ytl], axis=1)
        out[expert_tokens[e]] += yT[:, :cnt].T.astype(np.float32)
    return out
